# revision 4
# baseline (speedup 1.0000x reference)
"""Butterworth bandpass filtfilt on Trainium2 (8 NeuronCores).

Algorithm: the order-16 IIR filtfilt is numerically equivalent (to ~1e-6 rel)
to a truncated-FIR convolution because the slowest pole has radius 0.9808
(impulse response < 1e-7 after ~830 samples).  Each direction becomes 6
PSUM-accumulated block-Toeplitz [128x128] matmuls per 128-sample chunk:
  y1[c] = sum_d G_d @ x[c-d]   (forward,  G_d[j,m] = h[128d + j - m])
  y2[c] = sum_d G_d^T @ y1[c+d] (backward)
with scipy-filtfilt edge handling (odd extension + lfilter_zi constant
extension) folded into constant left/right padding and a per-clip
broadcast fill of y1's last value.

Data layout: batch is sharded 16 clips/core.  Host pre-transposes input to
[pos-in-chunk, chunk] (partition-major) fp16; taps are scaled by 4096 to
stay in fp16 normal range and descaled in the PSUM->SBUF copies.  The
output is un-transposed on-device via PE transposes.  Exactly two DMA
instructions per core (one packed input load, one strided store) keep the
Tile tail-drain within the walrus sync-wait-slot limit; the drain itself is
split into single-wait drains via the _drain_and_barrier patch below.
"""

import numpy as np

K = 128
D = 6
SCALE = 4096.0
PAD = 51
T = 160000
TEXT = T + 2 * PAD            # 160102
PL = (D - 1) * K              # 640 constant left pad
CLIPS = 16                    # per core
CA = 1264                     # input chunks per clip (mult of 16; 1264*128 >= PL+TEXT)
CB = 1256                     # y1 chunks per clip (1251 computed + 5 const)
NYC = 1251                    # valid output chunks per clip
NXC = CLIPS * CA              # 20224
NYB = CLIPS * CB              # 20096
NOUT = CLIPS * NYC            # 20016
NBLK = 157                    # ceil(NOUT/128); 157*128 = 20096
GCOLS = 12 * K                # 1536
XIN_COLS = NXC + GCOLS + K + K  # 22016

ORDER = 8
FS = 16000.0
LOWER = 300.0
UPPER = 3000.0


def _butter_bandpass(order, w1, w2):
    fs = 2.0
    warped = 2.0 * fs * np.tan(np.pi * np.array([w1, w2]) / fs)
    bw = warped[1] - warped[0]
    wo = np.sqrt(warped[0] * warped[1])
    k = np.arange(1, order + 1)
    p = np.exp(1j * np.pi * (2 * k + order - 1) / (2 * order))
    p_lp = p * (bw / 2.0)
    disc = np.sqrt(p_lp ** 2 - wo ** 2)
    p_bp = np.concatenate([p_lp + disc, p_lp - disc])
    z_bp = np.zeros(order, dtype=complex)
    k_bp = bw ** order
    fs2 = 2.0 * fs
    z_z = np.concatenate([(fs2 + z_bp) / (fs2 - z_bp), -np.ones(order)])
    p_z = (fs2 + p_bp) / (fs2 - p_bp)
    k_z = k_bp * np.real(np.prod(fs2 - z_bp) / np.prod(fs2 - p_bp))
    return np.real(k_z * np.poly(z_z)), np.real(np.poly(p_z))


def _impulse_response(b, a, L):
    n = len(a)
    z = np.zeros(n - 1)
    h = np.zeros(L)
    for t in range(L):
        xt = 1.0 if t == 0 else 0.0
        yt = b[0] * xt + z[0]
        z[:-1] = z[1:]
        z[-1] = 0.0
        z += b[1:] * xt - a[1:] * yt
        h[t] = yt
    return h


def _build_weights(b, a):
    h = _impulse_response(np.asarray(b, np.float64), np.asarray(a, np.float64), D * K + K)
    gf = []  # lhsT for forward: gf_d[m, j] = G_d[j, m] = h[dK + j - m]
    gb = []  # lhsT for backward: gb_d[m, j] = G_d[m, j] = h[dK + m - j]
    hh = np.zeros(D * K + K)
    hh[:len(h)] = h
    mm = np.arange(K)[:, None]
    jj = np.arange(K)[None, :]
    for d in range(D):
        tf = d * K + jj - mm
        tb = d * K + mm - jj
        Gf = np.where((tf >= 0) & (tf < len(hh)), hh[np.clip(tf, 0, len(hh) - 1)], 0.0)
        Gb = np.where((tb >= 0) & (tb < len(hh)), hh[np.clip(tb, 0, len(hh) - 1)], 0.0)
        gf.append(Gf)
        gb.append(Gb)
    gpack = np.concatenate(gf + gb, axis=1) * SCALE      # [128, 1536]
    sel = np.zeros((K, K))
    sel[101, :] = 1.0
    ident = np.eye(K)
    return np.concatenate([gpack, sel, ident], axis=1).astype(np.float16)  # [128, 1792]


def _build_bass():
    import concourse.bass as bass
    import concourse.mybir as mybir
    from concourse.tile import TileContext
    import concourse.tile as tile_mod
    from concourse.vector_clock import ScopedClock, VectorClock

    # walrus in this toolchain rejects instructions with >~3 sync waits; the
    # Tile tail drain waits on every proc lane in one instruction.  Split it
    # into single-wait drains.
    def _split_drain_and_barrier(self, tick_clock, wait_clock):
        gv = tick_clock.global_clock
        for i, t in enumerate(list(gv)):
            if t <= 0:
                continue
            sub = VectorClock()
            sub.require_at_least(i, t)
            d = self.nc.sync.drain()
            wait_clock.add_sem_waits(d.ins, ScopedClock({None: sub}))
        self.nc.all_engine_barrier()
        assert self.sems is not None
        popped = self.nc._tile_sem_poison_stack.pop()
        assert popped is self._sem_poison
        self.nc.clear_and_free_semaphores(list(self.sems.allocated().values()))
        self.nc.all_engine_barrier()

    tile_mod.TileContext._drain_and_barrier = _split_drain_and_barrier

    F16 = mybir.dt.float16
    F32 = mybir.dt.float32

    nc = bass.Bass()
    xin = nc.dram_tensor("xin", [K, XIN_COLS], F16, kind="ExternalInput")
    yout = nc.dram_tensor("y", [NBLK * K, K], F16, kind="ExternalOutput")

    jobs = [(0, 512), (512, 512), (1024, NYC - 1024)]

    with TileContext(nc) as tc:
        with (
            tc.tile_pool(name="big", bufs=1) as big,
            tc.tile_pool(name="ps", bufs=5, space="PSUM") as psp,
            tc.tile_pool(name="pb", bufs=1, space="PSUM") as pbp,
            tc.tile_pool(name="pt", bufs=2, space="PSUM") as ptp,
        ):
            allb = big.tile([K, XIN_COLS], F16, tag="allb")
            y1t = big.tile([K, NYB], F16, tag="y1t")
            y2t = big.tile([K, NBLK * K], F16, tag="y2t")
            nat = big.tile([K, NBLK * K], F16, tag="nat")

            nc.sync.dma_start(out=allb[:, :], in_=xin[:, :])
            XT = allb[:, 0:NXC]
            GG = allb[:, NXC:NXC + GCOLS]
            SEL = allb[:, NXC + GCOLS:NXC + GCOLS + K]
            IDT = allb[:, NXC + GCOLS + K:NXC + GCOLS + 2 * K]

            def gf(d):
                return GG[:, d * K:(d + 1) * K]

            def gb(d):
                return GG[:, (D + d) * K:(D + d + 1) * K]

            # forward pass + per-clip constant fill of y1 tail
            for bcl in range(CLIPS):
                xb = bcl * CA
                yb = bcl * CB
                ps_last = None
                for c0, w in jobs:
                    ps = psp.tile([K, 512], F32, tag="ps")
                    for d in range(D):
                        s0 = xb + c0 + (D - 1) - d
                        nc.tensor.matmul(ps[:, :w], gf(d), XT[:, s0:s0 + w],
                                         start=(d == 0), stop=(d == D - 1))
                    nc.scalar.mul(y1t[:, yb + c0:yb + c0 + w], ps[:, :w], 1.0 / SCALE)
                    ps_last = (ps, w)
                pb = pbp.tile([K, 1], F32, tag="pb")
                nc.tensor.matmul(pb[:, :], SEL, y1t[:, yb + 1250:yb + 1251],
                                 start=True, stop=True)
                for c in range(NYC, CB):
                    nc.scalar.mul(y1t[:, yb + c:yb + c + 1], pb[:, :], 1.0)
                # ext positions 160102..160127 live in rows 102..127 of chunk
                # 1250; partition bases must be aligned, so overwrite the whole
                # column with the const then restore rows 0..101 from PSUM.
                ps3, w3 = ps_last
                nc.scalar.mul(y1t[:, yb + 1250:yb + 1251], pb[:, :], 1.0)
                nc.scalar.mul(y1t[0:102, yb + 1250:yb + 1251],
                              ps3[0:102, w3 - 1:w3], 1.0 / SCALE)

            nc.gpsimd.memset(y2t[:, NOUT:], 0.0)

            # backward pass
            for bcl in range(CLIPS):
                yb = bcl * CB
                zb = bcl * NYC
                for c0, w in jobs:
                    ps = psp.tile([K, 512], F32, tag="ps")
                    for d in range(D):
                        s0 = yb + c0 + d
                        nc.tensor.matmul(ps[:, :w], gb(d), y1t[:, s0:s0 + w],
                                         start=(d == 0), stop=(d == D - 1))
                    nc.scalar.mul(y2t[:, zb + c0:zb + c0 + w], ps[:, :w], 1.0 / SCALE)

            # un-transpose via PE, then one strided store
            for s in range(NBLK):
                pt = ptp.tile([K, K], F16, tag="pt")
                nc.tensor.transpose(pt[:, :], y2t[:, s * K:(s + 1) * K], IDT)
                nc.scalar.mul(nat[:, s * K:(s + 1) * K], pt[:, :], 1.0)

            yv = yout.rearrange("(s cb) j -> cb s j", cb=K)
            nv = nat[:, :].rearrange("p (s j) -> p s j", j=K)
            nc.sync.dma_start(out=yv, in_=nv)
    return nc


_NC_CACHE = None


def kernel(audio, b=None, a=None, _want_results_obj=False, _trace=False):
    global _NC_CACHE
    from concourse.bass_utils import run_bass_kernel_spmd

    audio = np.asarray(audio)
    B = audio.shape[0]
    assert audio.shape == (128, T), audio.shape
    if b is None or a is None:
        b, a = _butter_bandpass(ORDER, 2 * LOWER / FS, 2 * UPPER / FS)
    b = np.asarray(b, np.float64)
    a = np.asarray(a, np.float64)

    consts = _build_weights(b, a)                    # [128, 1792] fp16

    # host prep: odd extension + constant pads, fp16, pos-major transpose
    x = audio.astype(np.float64)
    left = 2.0 * x[:, :1] - x[:, 1:PAD + 1][:, ::-1]
    right = 2.0 * x[:, -1:] - x[:, -PAD - 1:-1][:, ::-1]
    A = np.empty((B, CA * K), np.float16)
    A[:, :PL] = left[:, :1].astype(np.float16)       # const ext[0] == left[0]
    A[:, PL:PL + PAD] = left.astype(np.float16)
    A[:, PL + PAD:PL + PAD + T] = audio.astype(np.float16)
    A[:, PL + PAD + T:PL + TEXT] = right.astype(np.float16)
    A[:, PL + TEXT:] = right[:, -1:].astype(np.float16)
    # [B, CA, K] -> [B, K, CA]
    At = np.ascontiguousarray(A.reshape(B, CA, K).transpose(0, 2, 1))

    n_cores = 8
    per = B // n_cores
    in_maps = []
    for c in range(n_cores):
        xc = At[c * per:(c + 1) * per]               # [16, 128, CA]
        xin = np.empty((K, XIN_COLS), np.float16)
        xin[:, :NXC] = xc.transpose(1, 0, 2).reshape(K, NXC)
        xin[:, NXC:] = consts
        in_maps.append({"xin": xin})

    if _NC_CACHE is None:
        _NC_CACHE = _build_bass()
    import time as _time
    _t0 = _time.time()
    res = run_bass_kernel_spmd(_NC_CACHE, in_maps, core_ids=list(range(n_cores)),
                               trace=_trace)
    res.run_wall_s = _time.time() - _t0

    out = np.empty((B, T), np.float64)
    for c in range(n_cores):
        yc = res.results[c]["y"][:NOUT].astype(np.float64)     # [20016, 128]
        yc = yc.reshape(per, NYC * K)
        out[c * per:(c + 1) * per] = yc[:, PAD:PAD + T]
    if _want_results_obj:
        return out, res
    return out


if __name__ == "__main__":
    rng = np.random.default_rng(0)
    audio = rng.standard_normal((128, T)).astype(np.float32)
    y = kernel(audio)
    print("ran:", y.shape, y.dtype, float(np.abs(y).max()))
